# revision 12
# baseline (speedup 1.0000x reference)
"""Trainium2 Bass kernel for nn_EnhancedMaskDecoder (MLM head + edge/GNN head).

Decomposition across 8 NeuronCores:
  - MLM vocab projection: vocab-sharded (each core owns ~V/8 embedding rows);
    per-shard CE stats (sum of exp) computed on-device, combined on host.
  - Edge path: data-parallel over the extracted edge list (contiguous shards).
  - The small head transform of the masked tokens is replicated on all cores.

Host side (this file, numpy): nonzero/gather index extraction, weight folds
(LN gain folded into embedding matrices, LN bias folded into logit biases),
sharding/padding, upload, and final assembly (concat + CE from per-shard
stats).  Device side (Bass/Tile): all matmuls, gelu, layernorm, exp/log-
sum-exp.  Matmuls run as float32r (full PE rate, ~1e-4 relative rounding).
"""
import os
import sys

sys.path.insert(0, "/opt/trn_rl_repo")

import numpy as np

import concourse.bacc as bacc
import concourse.mybir as mybir
import concourse.tile as tile
from concourse import bass_utils
from concourse.bass import ds, ts
from concourse.masks import make_identity

N_CORES = 8
H = 1024
LN_EPS = 1e-7
LASSO = 1e-4
NEG = -1e30

F32 = mybir.dt.float32
F32R = mybir.dt.float32r
AF = mybir.ActivationFunctionType
ALU = mybir.AluOpType
AX = mybir.AxisListType


def _build_program(MP, KP, VP):
    """One SPMD program; all shapes identical across cores.

    Inputs (float32r unless noted):
      ctxT   [H, MP]     masked-token hidden states, transposed
      lmwT   [H, H]      lm_dense_w.T
      lmdb   [1, H]      lm_dense_b row
      EcT    [H, VP]     (ebd_weight * lm_ln_g).T vocab shard (zero-padded)
      bc     [1, VP]     per-shard logit bias row (NEG in padded columns)
      embT   [2H, KP]    edge embedding shard, transposed (zero-padded)
      gmwT   [2H, H]     gm_dense_w.T
      gmdb   [1, H]      gm_dense_b row
      geT    [H, 64]     (edge_ebd_weight * gm_ln_g).T
      gb     [1, 64]     edge logit bias row
    Outputs (float32):
      lm_logits [MP, VP], lm_sumexp [MP, 1], gm_logits [KP, 64], gm_lse [KP, 1]
    """
    MPT, KPT, VPT = MP // 128, KP // 128, VP // 512
    H2 = 2 * H
    HC = H // 128    # 8 contraction chunks for H
    H2C = H2 // 128  # 16 chunks for 2H

    nc = bacc.Bacc("TRN2", target_bir_lowering=False, debug=False)

    ctxT = nc.dram_tensor("ctxT", [H, MP], F32R, kind="ExternalInput")
    lmwT = nc.dram_tensor("lmwT", [H, H], F32R, kind="ExternalInput")
    lmdb = nc.dram_tensor("lmdb", [1, H], F32R, kind="ExternalInput")
    EcT = nc.dram_tensor("EcT", [H, VP], F32R, kind="ExternalInput")
    bc = nc.dram_tensor("bc", [1, VP], F32R, kind="ExternalInput")
    embT = nc.dram_tensor("embT", [H2, KP], F32R, kind="ExternalInput")
    gmwT = nc.dram_tensor("gmwT", [H2, H], F32R, kind="ExternalInput")
    gmdb = nc.dram_tensor("gmdb", [1, H], F32R, kind="ExternalInput")
    geT = nc.dram_tensor("geT", [H, 64], F32R, kind="ExternalInput")
    gb = nc.dram_tensor("gb", [1, 64], F32R, kind="ExternalInput")
    ones = nc.dram_tensor("ones", [1, 128], F32R, kind="ExternalInput")

    lm_logits = nc.dram_tensor("lm_logits", [MP, VP], F32, kind="ExternalOutput")
    lm_sumexp = nc.dram_tensor("lm_sumexp", [MP, 1], F32, kind="ExternalOutput")
    gm_logits = nc.dram_tensor("gm_logits", [KP, 64], F32, kind="ExternalOutput")
    gm_lse = nc.dram_tensor("gm_lse", [KP, 1], F32, kind="ExternalOutput")

    with tile.TileContext(nc) as tc:
        with tc.tile_pool(name="singles", bufs=1) as singles:
            ident = singles.tile([128, 128], F32)
            make_identity(nc, ident)
            magic = singles.tile([128, 1], mybir.dt.int32)
            nc.vector.memset(magic, 0x5F3759DF)
            ones1 = singles.tile([1, 128], F32R)
            nc.sync.dma_start(ones1, ones[:, :])
            TT = singles.tile([128, HC, MP], F32R)          # head output, transposed
            geT_sb = singles.tile([128, HC, 64], F32R)
            nc.sync.dma_start(geT_sb, geT[:, :].rearrange("(c p) v -> p c v", p=128))
            gb_sb = singles.tile([1, 64], F32R)
            nc.sync.dma_start(gb_sb, gb[:, :])
            separts = singles.tile([128, MPT, VPT], F32)    # sum-exp partials

            def newton_rsqrt(pool, rs, v):
                """rs = 1/sqrt(v) elementwise via bit-hack seed + 3 Newton
                steps.  DVE-only, so the scalar engine never has to swap its
                activation-table set away from gelu/exp."""
                i32 = mybir.dt.int32
                seed = pool.tile(list(rs.shape), F32, tag="nr_seed")
                nc.vector.tensor_scalar(
                    seed.bitcast(i32), v.bitcast(i32), 1, None,
                    op0=ALU.arith_shift_right)
                nc.vector.tensor_tensor(
                    seed.bitcast(i32), magic[: rs.shape[0]], seed.bitcast(i32),
                    mybir.AluOpType.subtract)
                y = rs
                nc.vector.tensor_copy(y, seed)
                a = pool.tile(list(rs.shape), F32, tag="nr_a")
                for _ in range(3):
                    nc.vector.tensor_mul(a, y, y)
                    nc.vector.tensor_mul(a, a, v)
                    nc.vector.tensor_scalar(a, a, -0.5, 1.5,
                                            op0=ALU.mult, op1=ALU.add)
                    nc.vector.tensor_mul(y, y, a)

            def layer_norm_normalize(pool, x_tile):
                """x (128, H) f32 -> (x - mean) * rsqrt(var + eps), in place."""
                nsg = H // 512
                stats = pool.tile([128, nsg, 6], F32, tag="ln_stats")
                for sg in range(nsg):
                    nc.vector.bn_stats(stats[:, sg], x_tile[:, ds(sg * 512, 512)])
                mv = pool.tile([128, 2], F32, tag="ln_mv")
                nc.vector.bn_aggr(mv, stats)
                ve = pool.tile([128, 1], F32, tag="ln_ve")
                nc.vector.tensor_scalar(ve, mv[:, 1:2], LN_EPS, None, op0=ALU.add)
                rs = pool.tile([128, 1], F32, tag="ln_rs")
                newton_rsqrt(pool, rs, ve)
                nc.vector.tensor_scalar(
                    x_tile, x_tile, mv[:, 0:1], rs, op0=ALU.subtract, op1=ALU.mult
                )

            # ---------------- Phase A: MLM head transform ----------------
            with tc.tile_pool(name="phA", bufs=2) as pA, \
                 tc.tile_pool(name="phA_ps", bufs=4, space="PSUM") as pAp, \
                 tc.tile_pool(name="phA_pt", bufs=2, space="PSUM") as pAt, \
                 tc.tile_pool(name="phA_w", bufs=1) as pAw:
                ctxT_sb = pAw.tile([128, HC, MP], F32R)
                nc.sync.dma_start(
                    ctxT_sb, ctxT[:, :].rearrange("(c p) m -> p c m", p=128))
                lmwT_sb = pAw.tile([128, HC, H], F32R)
                nc.sync.dma_start(
                    lmwT_sb, lmwT[:, :].rearrange("(c p) h -> p c h", p=128))
                lmdb_sb = pAw.tile([1, H], F32R)
                nc.sync.dma_start(lmdb_sb, lmdb[:, :])

                for mt in range(MPT):
                    t_tile = pA.tile([128, H], F32, tag="t_tile")
                    for half in range(2):
                        ps = pAp.tile([128, 512], F32, tag="headps")
                        for k in range(HC):
                            nc.tensor.matmul(
                                ps, ctxT_sb[:, k, ts(mt, 128)],
                                lmwT_sb[:, k, ds(half * 512, 512)],
                                start=(k == 0), stop=False)
                        nc.tensor.matmul(
                            ps, ones1, lmdb_sb[:, ds(half * 512, 512)],
                            start=False, stop=True)
                        nc.scalar.activation(
                            t_tile[:, ds(half * 512, 512)], ps, AF.Gelu)
                    layer_norm_normalize(pA, t_tile)
                    for k in range(HC):
                        pt = pAt.tile([128, 128], F32, tag="tps")
                        nc.tensor.transpose(pt, t_tile[:, ts(k, 128)], ident)
                        nc.vector.tensor_copy(TT[:, k, ts(mt, 128)], pt)

            # ---------------- Phase B: edge path ----------------
            with tc.tile_pool(name="phB", bufs=3) as pB, \
                 tc.tile_pool(name="phB_ps", bufs=4, space="PSUM") as pBp, \
                 tc.tile_pool(name="phB_pt", bufs=2, space="PSUM") as pBt, \
                 tc.tile_pool(name="phB_pg", bufs=2, space="PSUM") as pBg, \
                 tc.tile_pool(name="phB_w", bufs=1) as pBw:
                gmwT_sb = pBw.tile([128, H2C, H], F32R)
                nc.sync.dma_start(
                    gmwT_sb, gmwT[:, :].rearrange("(c p) h -> p c h", p=128))
                gmdb_sb = pBw.tile([1, H], F32R)
                nc.sync.dma_start(gmdb_sb, gmdb[:, :])
                embT_view = embT[:, :].rearrange("(c p) k -> p c k", p=128)
                glog_all = pBw.tile([128, KPT, 64], F32)
                gse_all = pBw.tile([128, KPT], F32)

                for kt in range(KPT):
                    emb_t = pB.tile([128, H2C, 128], F32R, tag="emb_t")
                    nc.sync.dma_start(emb_t, embT_view[:, :, ts(kt, 128)])
                    g_tile = pB.tile([128, H], F32, tag="g_tile")
                    for half in range(2):
                        ps = pBp.tile([128, 512], F32, tag="edgeps")
                        for e in range(H2C):
                            nc.tensor.matmul(
                                ps, emb_t[:, e, :],
                                gmwT_sb[:, e, ds(half * 512, 512)],
                                start=(e == 0), stop=False)
                        nc.tensor.matmul(
                            ps, ones1, gmdb_sb[:, ds(half * 512, 512)],
                            start=False, stop=True)
                        nc.scalar.activation(
                            g_tile[:, ds(half * 512, 512)], ps, AF.Gelu)
                    layer_norm_normalize(pB, g_tile)
                    gT = pB.tile([128, HC, 128], F32R, tag="gT")
                    for k in range(HC):
                        pt = pBt.tile([128, 128], F32, tag="gps")
                        nc.tensor.transpose(pt, g_tile[:, ts(k, 128)], ident)
                        nc.vector.tensor_copy(gT[:, k, :], pt)
                    pg = pBg.tile([128, 64], F32, tag="glps")
                    for k in range(HC):
                        nc.tensor.matmul(pg, gT[:, k, :], geT_sb[:, k, :],
                                         start=(k == 0), stop=False)
                    nc.tensor.matmul(pg, ones1, gb_sb, start=False, stop=True)
                    nc.vector.tensor_copy(glog_all[:, kt, :], pg)
                    nc.sync.dma_start(gm_logits[ts(kt, 128), :],
                                      glog_all[:, kt, :])

                # batched exp + log-sum-exp (one activation-table set, no
                # per-tile gelu<->exp table swaps)
                for kt in range(KPT):
                    gexp = pB.tile([128, 64], F32, tag="gexp")
                    nc.scalar.activation(gexp, glog_all[:, kt, :], AF.Exp,
                                         accum_out=gse_all[:, kt:kt + 1])
                lse_all = pBw.tile([128, KPT], F32)
                nc.scalar.activation(lse_all, gse_all, AF.Ln)
                for kt in range(KPT):
                    nc.sync.dma_start(gm_lse[ts(kt, 128), :],
                                      lse_all[:, kt:kt + 1])

            # ---------------- Phase C: vocab projection ----------------
            with tc.tile_pool(name="phC", bufs=3) as pC, \
                 tc.tile_pool(name="phC_o", bufs=4) as pCo, \
                 tc.tile_pool(name="phC_w", bufs=1) as pCw, \
                 tc.tile_pool(name="phC_ps", bufs=4, space="PSUM") as pCp:
                bc_sb = pCw.tile([1, VP], F32R)
                nc.sync.dma_start(bc_sb, bc[:, :])
                EcT_view = EcT[:, :].rearrange("(c p) v -> p c v", p=128)
                for vt in range(VPT):
                    e_t = pC.tile([128, HC, 512], F32R, tag="e_t")
                    nc.sync.dma_start(e_t, EcT_view[:, :, ts(vt, 512)])
                    for mt in range(MPT):
                        ps = pCp.tile([128, 512], F32, tag="vps")
                        for k in range(HC):
                            nc.tensor.matmul(ps, TT[:, k, ts(mt, 128)],
                                             e_t[:, k, :],
                                             start=(k == 0), stop=False)
                        nc.tensor.matmul(ps, ones1, bc_sb[:, ds(vt * 512, 512)],
                                         start=False, stop=True)
                        lg = pCo.tile([128, 512], F32, tag="lg")
                        nc.vector.tensor_copy(lg, ps)
                        ex = pCo.tile([128, 512], F32, tag="ex")
                        nc.scalar.activation(ex, ps, AF.Exp,
                                             accum_out=separts[:, mt, vt:vt + 1])
                        nc.sync.dma_start(
                            lm_logits[ts(mt, 128), ds(vt * 512, 512)], lg)
                # row-sum the partials and write out
                for mt in range(MPT):
                    se = pC.tile([128, 1], F32, tag="se")
                    nc.vector.tensor_reduce(se, separts[:, mt, :], axis=AX.X,
                                            op=ALU.add)
                    nc.sync.dma_start(lm_sumexp[ts(mt, 128), :], se)

    nc.compile()
    return nc


_PROGRAM_CACHE = {}


def _get_program(MP, KP, VP):
    key = (MP, KP, VP)
    if key not in _PROGRAM_CACHE:
        _PROGRAM_CACHE[key] = _build_program(MP, KP, VP)
    return _PROGRAM_CACHE[key]


def _pad128(n):
    return max(128, -(-n // 128) * 128)


def kernel(hidden, ebd_weight, edge_ebd_weight, lm_dense_w, lm_dense_b, lm_ln_g,
           lm_ln_b, lm_bias, gm_dense_w, gm_dense_b, gm_ln_g, gm_ln_b, gm_bias,
           target_ids, adj_matrix, _trace=False):
    hidden = np.asarray(hidden, np.float32)
    B, L, Hd = hidden.shape
    V = ebd_weight.shape[0]
    EV = edge_ebd_weight.shape[0]
    assert Hd == H

    # ---------------- host prep: MLM ----------------
    flat_t = np.asarray(target_ids).reshape(-1)
    mask_idx = np.nonzero(flat_t > 0)[0]
    M = len(mask_idx)
    MP = _pad128(M)
    flat_h = hidden.reshape(-1, H)
    ctx = np.zeros((MP, H), np.float32)
    ctx[:M] = flat_h[mask_idx]
    lm_labels = flat_t[mask_idx].astype(np.int32)

    vs = -(-V // N_CORES)
    bounds = [(c * vs, min((c + 1) * vs, V)) for c in range(N_CORES)]
    VP = -(-vs // 512) * 512

    E_eff = (np.asarray(ebd_weight, np.float32)
             * np.asarray(lm_ln_g, np.float32)[None, :])
    # logits = (n*g) @ E.T + b @ E.T + lm_bias; fold g into E, b into the bias
    lm_bias_eff = (np.asarray(lm_bias, np.float32)
                   + np.asarray(ebd_weight, np.float32)
                   @ np.asarray(lm_ln_b, np.float32)).astype(np.float32)

    # ---------------- host prep: edges ----------------
    adj = np.asarray(adj_matrix)
    nz = np.stack(np.nonzero(adj), axis=1)
    non_diag = nz[:, 1] != nz[:, 2]
    trans = np.stack([nz[:, 0], nz[:, 2], nz[:, 1]], axis=1)[non_diag]
    nz_all = np.concatenate([nz, trans], axis=0)
    K2 = len(nz_all)
    edge_labels = adj[nz_all[:, 0], nz_all[:, 1], nz_all[:, 2]].astype(np.int32)
    eflat = nz_all[:, 0] * L
    edge_to = flat_h[eflat + nz_all[:, 2]]
    edge_from = flat_h[eflat + nz_all[:, 1]]

    splits = np.array_split(np.arange(K2), N_CORES)
    KP = _pad128(max(len(s) for s in splits))

    Ge_eff = (np.asarray(edge_ebd_weight, np.float32)
              * np.asarray(gm_ln_g, np.float32)[None, :])
    gm_bias_eff = (np.asarray(gm_bias, np.float32)
                   + np.asarray(edge_ebd_weight, np.float32)
                   @ np.asarray(gm_ln_b, np.float32)).astype(np.float32)

    lasso = np.float32(LASSO * np.abs(np.asarray(gm_dense_w, np.float64)).sum())

    # ---------------- shared uploads ----------------
    ctxT = np.ascontiguousarray(ctx.T)
    lmwT = np.ascontiguousarray(np.asarray(lm_dense_w, np.float32).T)
    lmdb = np.asarray(lm_dense_b, np.float32).reshape(1, H)
    gmwT = np.ascontiguousarray(np.asarray(gm_dense_w, np.float32).T)
    gmdb = np.asarray(gm_dense_b, np.float32).reshape(1, H)
    geT = np.zeros((H, 64), np.float32)
    geT[:, :EV] = Ge_eff.T
    gbrow = np.full((1, 64), NEG, np.float32)
    gbrow[0, :EV] = gm_bias_eff

    in_maps = []
    for c in range(N_CORES):
        v0, v1 = bounds[c]
        EcT_c = np.zeros((H, VP), np.float32)
        EcT_c[:, :v1 - v0] = E_eff[v0:v1].T
        bc_c = np.full((1, VP), NEG, np.float32)
        bc_c[0, :v1 - v0] = lm_bias_eff[v0:v1]
        idx = splits[c]
        k = len(idx)
        embT_c = np.zeros((2 * H, KP), np.float32)
        embT_c[:H, :k] = edge_to[idx].T
        embT_c[H:, :k] = edge_from[idx].T
        in_maps.append({
            "ctxT": ctxT, "lmwT": lmwT, "lmdb": lmdb,
            "EcT": np.ascontiguousarray(EcT_c), "bc": bc_c,
            "embT": np.ascontiguousarray(embT_c), "gmwT": gmwT, "gmdb": gmdb,
            "geT": geT, "gb": gbrow,
            "ones": np.ones((1, 128), np.float32),
        })

    # ---------------- run on the 8 cores ----------------
    nc = _get_program(MP, KP, VP)
    res = bass_utils.run_bass_kernel_spmd(
        nc, in_maps, core_ids=list(range(N_CORES)), trace=_trace)

    # ---------------- host assembly ----------------
    lm_logits = np.empty((M, V), np.float32)
    sumexp = np.zeros(M, np.float64)
    for c in range(N_CORES):
        v0, v1 = bounds[c]
        r = res.results[c]
        lm_logits[:, v0:v1] = r["lm_logits"][:M, :v1 - v0]
        sumexp += r["lm_sumexp"][:M, 0].astype(np.float64)
    logZ = np.log(sumexp).astype(np.float32)
    lm_loss = (logZ - lm_logits[np.arange(M), lm_labels]).astype(np.float32)

    gm_logits = np.empty((K2, EV), np.float32)
    gm_lse = np.empty(K2, np.float32)
    for c in range(N_CORES):
        idx = splits[c]
        k = len(idx)
        r = res.results[c]
        if k:
            gm_logits[idx[0]:idx[0] + k] = r["gm_logits"][:k, :EV]
            gm_lse[idx[0]:idx[0] + k] = r["gm_lse"][:k, 0]
    gm_loss = (gm_lse - gm_logits[np.arange(K2), edge_labels] + lasso
               ).astype(np.float32)

    out = (lm_logits, lm_labels, lm_loss, gm_logits, gm_loss, edge_labels)
    if _trace:
        return out, res
    return out


# revision 13
# speedup vs baseline: 1.1310x; 1.1310x over previous
"""Trainium2 Bass kernel for nn_EnhancedMaskDecoder (MLM head + edge/GNN head).

Decomposition across 8 NeuronCores:
  - MLM vocab projection: vocab-sharded (each core owns ~V/8 embedding rows);
    per-shard CE stats (sum of exp) computed on-device, combined on host.
  - Edge path: data-parallel over the extracted edge list (contiguous shards).
  - The small head transform of the masked tokens is replicated on all cores.

Host side (this file, numpy): nonzero/gather index extraction, weight folds
(LN gain folded into embedding matrices, LN bias folded into logit biases),
sharding/padding, upload, and final assembly (concat + CE from per-shard
stats).  Device side (Bass/Tile): all matmuls, gelu, layernorm, exp/log-
sum-exp.  Matmuls run as float32r (full PE rate, ~1e-4 relative rounding).
"""
import os
import sys

sys.path.insert(0, "/opt/trn_rl_repo")

import numpy as np

import concourse.bacc as bacc
import concourse.mybir as mybir
import concourse.tile as tile
from concourse import bass_utils
from concourse.bass import ds, ts
from concourse.masks import make_identity

N_CORES = 8
H = 1024
LN_EPS = 1e-7
LASSO = 1e-4
NEG = -1e30

F32 = mybir.dt.float32
F32R = mybir.dt.float32r
AF = mybir.ActivationFunctionType
ALU = mybir.AluOpType
AX = mybir.AxisListType


def _build_program(MP, KP, VP, VW, has_lmdb, has_gmdb, has_lmb, has_gmb):
    """One SPMD program; all shapes identical across cores.

    VW: exact vocab width for the sum-exp (pad columns beyond VW are never
    exp'd; pad columns inside VW are corrected host-side when has_lmb=False).
    has_*: emit the K=1 ones-row bias matmuls only when the bias is nonzero.
    """
    MPT, KPT, VPT = MP // 128, KP // 128, VP // 512
    H2 = 2 * H
    HC = H // 128
    H2C = H2 // 128

    nc = bacc.Bacc("TRN2", target_bir_lowering=False, debug=False)

    ctxT = nc.dram_tensor("ctxT", [H, MP], F32R, kind="ExternalInput")
    lmwT = nc.dram_tensor("lmwT", [H, H], F32R, kind="ExternalInput")
    EcT = nc.dram_tensor("EcT", [H, VP], F32R, kind="ExternalInput")
    embT = nc.dram_tensor("embT", [H2, KP], F32R, kind="ExternalInput")
    gmwT = nc.dram_tensor("gmwT", [H2, H], F32R, kind="ExternalInput")
    geT = nc.dram_tensor("geT", [H, 64], F32R, kind="ExternalInput")
    need_ones = has_lmdb or has_gmdb or has_lmb or has_gmb
    if need_ones:
        ones = nc.dram_tensor("ones", [1, 128], F32R, kind="ExternalInput")
    if has_lmdb:
        lmdb = nc.dram_tensor("lmdb", [1, H], F32R, kind="ExternalInput")
    if has_gmdb:
        gmdb = nc.dram_tensor("gmdb", [1, H], F32R, kind="ExternalInput")
    if has_lmb:
        bc = nc.dram_tensor("bc", [1, VP], F32R, kind="ExternalInput")
    if has_gmb:
        gb = nc.dram_tensor("gb", [1, 64], F32R, kind="ExternalInput")

    lm_logits = nc.dram_tensor("lm_logits", [MP, VP], F32, kind="ExternalOutput")
    lm_sumexp = nc.dram_tensor("lm_sumexp", [MP, 1], F32, kind="ExternalOutput")
    gm_logits = nc.dram_tensor("gm_logits", [KP, 64], F32, kind="ExternalOutput")
    gm_lse = nc.dram_tensor("gm_lse", [KP, 1], F32, kind="ExternalOutput")

    with tile.TileContext(nc) as tc:
        with tc.tile_pool(name="singles", bufs=1) as singles:
            ident = singles.tile([128, 128], F32)
            make_identity(nc, ident)
            magic = singles.tile([128, 1], mybir.dt.int32)
            nc.vector.memset(magic, 0x5F3759DF)
            if need_ones:
                ones1 = singles.tile([1, 128], F32R)
                nc.sync.dma_start(ones1, ones[:, :])
            TT = singles.tile([128, HC, MP], F32R)
            geT_sb = singles.tile([128, HC, 64], F32R)
            nc.sync.dma_start(geT_sb, geT[:, :].rearrange("(c p) v -> p c v", p=128))
            if has_gmb:
                gb_sb = singles.tile([1, 64], F32R)
                nc.sync.dma_start(gb_sb, gb[:, :])
            separts = singles.tile([128, MPT, VPT], F32)

            def newton_rsqrt(pool, rs, v):
                """rs = 1/sqrt(v) via bit-hack seed + 3 Newton steps; DVE-only
                so the scalar engine never swaps its activation-table set."""
                i32 = mybir.dt.int32
                seed = pool.tile(list(rs.shape), F32, tag="nr_seed")
                nc.vector.tensor_scalar(
                    seed.bitcast(i32), v.bitcast(i32), 1, None,
                    op0=ALU.arith_shift_right)
                nc.vector.tensor_tensor(
                    seed.bitcast(i32), magic[: rs.shape[0]], seed.bitcast(i32),
                    mybir.AluOpType.subtract)
                y = rs
                nc.vector.tensor_copy(y, seed)
                a = pool.tile(list(rs.shape), F32, tag="nr_a")
                for _ in range(3):
                    nc.vector.tensor_mul(a, y, y)
                    nc.vector.tensor_mul(a, a, v)
                    nc.vector.tensor_scalar(a, a, -0.5, 1.5,
                                            op0=ALU.mult, op1=ALU.add)
                    nc.vector.tensor_mul(y, y, a)

            def layer_norm_normalize(pool, x_tile):
                """x (128, H) f32 -> (x - mean) * rsqrt(var + eps), in place."""
                nsg = H // 512
                stats = pool.tile([128, nsg, 6], F32, tag="ln_stats")
                for sg in range(nsg):
                    nc.vector.bn_stats(stats[:, sg], x_tile[:, ds(sg * 512, 512)])
                mv = pool.tile([128, 2], F32, tag="ln_mv")
                nc.vector.bn_aggr(mv, stats)
                ve = pool.tile([128, 1], F32, tag="ln_ve")
                nc.vector.tensor_scalar(ve, mv[:, 1:2], LN_EPS, None, op0=ALU.add)
                rs = pool.tile([128, 1], F32, tag="ln_rs")
                newton_rsqrt(pool, rs, ve)
                nc.vector.tensor_scalar(
                    x_tile, x_tile, mv[:, 0:1], rs, op0=ALU.subtract, op1=ALU.mult
                )

            # long-lived: edge-head weights preloaded at program start so the
            # DMA overlaps phase A compute (no pool-WAR stall entering B)
            with tc.tile_pool(name="gmw", bufs=1) as gmw:
                gmwT_sb = gmw.tile([128, H2C, H], F32R)
                nc.sync.dma_start(
                    gmwT_sb, gmwT[:, :].rearrange("(c p) h -> p c h", p=128))
                if has_gmdb:
                    gmdb_sb = gmw.tile([1, H], F32R)
                    nc.sync.dma_start(gmdb_sb, gmdb[:, :])
                glog_all = gmw.tile([128, KPT, 64], F32)
                gse_all = gmw.tile([128, KPT], F32)
                lse_all = gmw.tile([128, KPT], F32)

                # ---------------- Phase A: MLM head transform ----------------
                with tc.tile_pool(name="phA", bufs=3) as pA, \
                     tc.tile_pool(name="phA_ps", bufs=4, space="PSUM") as pAp, \
                     tc.tile_pool(name="phA_pt", bufs=2, space="PSUM") as pAt, \
                     tc.tile_pool(name="phA_w", bufs=1) as pAw:
                    lmwT_sb = pAw.tile([128, HC, H], F32R)
                    nc.sync.dma_start(
                        lmwT_sb, lmwT[:, :].rearrange("(c p) h -> p c h", p=128))
                    if has_lmdb:
                        lmdb_sb = pAw.tile([1, H], F32R)
                        nc.sync.dma_start(lmdb_sb, lmdb[:, :])
                    ctxT_view = ctxT[:, :].rearrange("(c p) m -> p c m", p=128)

                    for mt in range(MPT):
                        ctx_t = pA.tile([128, HC, 128], F32R, tag="ctx_t")
                        nc.sync.dma_start(ctx_t, ctxT_view[:, :, ts(mt, 128)])
                        t_tile = pA.tile([128, H], F32, tag="t_tile")
                        for half in range(2):
                            ps = pAp.tile([128, 512], F32, tag="headps")
                            for k in range(HC):
                                nc.tensor.matmul(
                                    ps, ctx_t[:, k, :],
                                    lmwT_sb[:, k, ds(half * 512, 512)],
                                    start=(k == 0),
                                    stop=(k == HC - 1 and not has_lmdb))
                            if has_lmdb:
                                nc.tensor.matmul(
                                    ps, ones1, lmdb_sb[:, ds(half * 512, 512)],
                                    start=False, stop=True)
                            nc.scalar.activation(
                                t_tile[:, ds(half * 512, 512)], ps, AF.Gelu)
                        layer_norm_normalize(pA, t_tile)
                        for k in range(HC):
                            pt = pAt.tile([128, 128], F32, tag="tps")
                            nc.tensor.transpose(pt, t_tile[:, ts(k, 128)], ident)
                            nc.vector.tensor_copy(TT[:, k, ts(mt, 128)], pt)

                # ---------------- Phase B: edge path ----------------
                with tc.tile_pool(name="phB", bufs=3) as pB, \
                     tc.tile_pool(name="phB2", bufs=2) as pB2, \
                     tc.tile_pool(name="phB_ps", bufs=4, space="PSUM") as pBp, \
                     tc.tile_pool(name="phB_pt", bufs=2, space="PSUM") as pBt, \
                     tc.tile_pool(name="phB_pg", bufs=2, space="PSUM") as pBg:
                    embT_view = embT[:, :].rearrange("(c p) k -> p c k", p=128)

                    for kt in range(KPT):
                        emb_t = pB.tile([128, H2C, 128], F32R, tag="emb_t")
                        nc.sync.dma_start(emb_t, embT_view[:, :, ts(kt, 128)])
                        g_tile = pB2.tile([128, H], F32, tag="g_tile")
                        for half in range(2):
                            ps = pBp.tile([128, 512], F32, tag="edgeps")
                            for e in range(H2C):
                                nc.tensor.matmul(
                                    ps, emb_t[:, e, :],
                                    gmwT_sb[:, e, ds(half * 512, 512)],
                                    start=(e == 0),
                                    stop=(e == H2C - 1 and not has_gmdb))
                            if has_gmdb:
                                nc.tensor.matmul(
                                    ps, ones1, gmdb_sb[:, ds(half * 512, 512)],
                                    start=False, stop=True)
                            nc.scalar.activation(
                                g_tile[:, ds(half * 512, 512)], ps, AF.Gelu)
                        layer_norm_normalize(pB2, g_tile)
                        gT = pB2.tile([128, HC, 128], F32R, tag="gT")
                        for k in range(HC):
                            pt = pBt.tile([128, 128], F32, tag="gps")
                            nc.tensor.transpose(pt, g_tile[:, ts(k, 128)], ident)
                            nc.vector.tensor_copy(gT[:, k, :], pt)
                        pg = pBg.tile([128, 64], F32, tag="glps")
                        for k in range(HC):
                            nc.tensor.matmul(pg, gT[:, k, :], geT_sb[:, k, :],
                                             start=(k == 0),
                                             stop=(k == HC - 1 and not has_gmb))
                        if has_gmb:
                            nc.tensor.matmul(pg, ones1, gb_sb,
                                             start=False, stop=True)
                        nc.vector.tensor_copy(glog_all[:, kt, :], pg)
                        nc.sync.dma_start(gm_logits[ts(kt, 128), :],
                                          glog_all[:, kt, :])

                    # batched exp + log-sum-exp (single activation-table set)
                    for kt in range(KPT):
                        gexp = pB2.tile([128, 64], F32, tag="gexp")
                        nc.scalar.activation(gexp, glog_all[:, kt, :], AF.Exp,
                                             accum_out=gse_all[:, kt:kt + 1])
                    nc.scalar.activation(lse_all, gse_all, AF.Ln)
                    for kt in range(KPT):
                        nc.sync.dma_start(gm_lse[ts(kt, 128), :],
                                          lse_all[:, kt:kt + 1])

            # ---------------- Phase C: vocab projection ----------------
            with tc.tile_pool(name="phC", bufs=3) as pC, \
                 tc.tile_pool(name="phC_o", bufs=4) as pCo, \
                 tc.tile_pool(name="phC_w", bufs=1) as pCw, \
                 tc.tile_pool(name="phC_ps", bufs=4, space="PSUM") as pCp:
                if has_lmb:
                    bc_sb = pCw.tile([1, VP], F32R)
                    nc.sync.dma_start(bc_sb, bc[:, :])
                EcT_view = EcT[:, :].rearrange("(c p) v -> p c v", p=128)
                for vt in range(VPT):
                    e_t = pC.tile([128, HC, 512], F32R, tag="e_t")
                    nc.sync.dma_start(e_t, EcT_view[:, :, ts(vt, 512)])
                    # exp over the exact vocab width only
                    ew = min(512, VW - vt * 512)
                    for mt in range(MPT):
                        ps = pCp.tile([128, 512], F32, tag="vps")
                        for k in range(HC):
                            nc.tensor.matmul(ps, TT[:, k, ts(mt, 128)],
                                             e_t[:, k, :],
                                             start=(k == 0),
                                             stop=(k == HC - 1 and not has_lmb))
                        if has_lmb:
                            nc.tensor.matmul(ps, ones1,
                                             bc_sb[:, ds(vt * 512, 512)],
                                             start=False, stop=True)
                        lg = pCo.tile([128, 512], F32, tag="lg")
                        nc.vector.tensor_copy(lg, ps)
                        if ew > 0:
                            ex = pCo.tile([128, 512], F32, tag="ex")
                            nc.scalar.activation(
                                ex[:, :ew], ps[:, :ew], AF.Exp,
                                accum_out=separts[:, mt, vt:vt + 1])
                        else:
                            nc.vector.memset(separts[:, mt, vt:vt + 1], 0.0)
                        nc.sync.dma_start(
                            lm_logits[ts(mt, 128), ds(vt * 512, 512)], lg)
                for mt in range(MPT):
                    se = pC.tile([128, 1], F32, tag="se")
                    nc.vector.tensor_reduce(se, separts[:, mt, :], axis=AX.X,
                                            op=ALU.add)
                    nc.sync.dma_start(lm_sumexp[ts(mt, 128), :], se)

    nc.compile()
    return nc


_PROGRAM_CACHE = {}


def _get_program(*key):
    if key not in _PROGRAM_CACHE:
        _PROGRAM_CACHE[key] = _build_program(*key)
    return _PROGRAM_CACHE[key]


def _pad128(n):
    return max(128, -(-n // 128) * 128)


def kernel(hidden, ebd_weight, edge_ebd_weight, lm_dense_w, lm_dense_b, lm_ln_g,
           lm_ln_b, lm_bias, gm_dense_w, gm_dense_b, gm_ln_g, gm_ln_b, gm_bias,
           target_ids, adj_matrix, _trace=False):
    hidden = np.asarray(hidden, np.float32)
    B, L, Hd = hidden.shape
    V = ebd_weight.shape[0]
    EV = edge_ebd_weight.shape[0]
    assert Hd == H

    # ---------------- host prep: MLM ----------------
    flat_t = np.asarray(target_ids).reshape(-1)
    mask_idx = np.nonzero(flat_t > 0)[0]
    M = len(mask_idx)
    MP = _pad128(M)
    flat_h = hidden.reshape(-1, H)
    ctx = np.zeros((MP, H), np.float32)
    ctx[:M] = flat_h[mask_idx]
    lm_labels = flat_t[mask_idx].astype(np.int32)

    vs = -(-V // N_CORES)
    bounds = [(c * vs, min((c + 1) * vs, V)) for c in range(N_CORES)]
    VP = -(-vs // 512) * 512
    VW = vs  # exp width used on device (max shard width)

    E_eff = (np.asarray(ebd_weight, np.float32)
             * np.asarray(lm_ln_g, np.float32)[None, :])
    # logits = (n*g) @ E.T + b @ E.T + lm_bias; fold g into E, b into the bias
    lm_bias_eff = (np.asarray(lm_bias, np.float32)
                   + np.asarray(ebd_weight, np.float32)
                   @ np.asarray(lm_ln_b, np.float32)).astype(np.float32)

    # ---------------- host prep: edges ----------------
    adj = np.asarray(adj_matrix)
    nz = np.stack(np.nonzero(adj), axis=1)
    non_diag = nz[:, 1] != nz[:, 2]
    trans = np.stack([nz[:, 0], nz[:, 2], nz[:, 1]], axis=1)[non_diag]
    nz_all = np.concatenate([nz, trans], axis=0)
    K2 = len(nz_all)
    edge_labels = adj[nz_all[:, 0], nz_all[:, 1], nz_all[:, 2]].astype(np.int32)
    eflat = nz_all[:, 0] * L
    edge_to = flat_h[eflat + nz_all[:, 2]]
    edge_from = flat_h[eflat + nz_all[:, 1]]

    splits = np.array_split(np.arange(K2), N_CORES)
    KP = _pad128(max(len(s) for s in splits))

    Ge_eff = (np.asarray(edge_ebd_weight, np.float32)
              * np.asarray(gm_ln_g, np.float32)[None, :])
    gm_bias_eff = (np.asarray(gm_bias, np.float32)
                   + np.asarray(edge_ebd_weight, np.float32)
                   @ np.asarray(gm_ln_b, np.float32)).astype(np.float32)

    lasso = np.float32(LASSO * np.abs(np.asarray(gm_dense_w, np.float64)).sum())

    lmdb = np.asarray(lm_dense_b, np.float32).reshape(1, H)
    gmdb = np.asarray(gm_dense_b, np.float32).reshape(1, H)
    has_lmdb = bool(np.any(lmdb))
    has_gmdb = bool(np.any(gmdb))
    has_lmb = bool(np.any(lm_bias_eff))
    has_gmb = bool(np.any(gm_bias_eff)) or EV < 64

    # ---------------- shared uploads ----------------
    ctxT = np.ascontiguousarray(ctx.T)
    lmwT = np.ascontiguousarray(np.asarray(lm_dense_w, np.float32).T)
    gmwT = np.ascontiguousarray(np.asarray(gm_dense_w, np.float32).T)
    geT = np.zeros((H, 64), np.float32)
    geT[:, :EV] = Ge_eff.T
    gbrow = np.full((1, 64), NEG, np.float32)
    gbrow[0, :EV] = gm_bias_eff

    in_maps = []
    for c in range(N_CORES):
        v0, v1 = bounds[c]
        EcT_c = np.zeros((H, VP), np.float32)
        EcT_c[:, :v1 - v0] = E_eff[v0:v1].T
        bc_c = np.full((1, VP), NEG, np.float32)
        bc_c[0, :v1 - v0] = lm_bias_eff[v0:v1]
        idx = splits[c]
        k = len(idx)
        embT_c = np.zeros((2 * H, KP), np.float32)
        embT_c[:H, :k] = edge_to[idx].T
        embT_c[H:, :k] = edge_from[idx].T
        in_maps.append({
            "ctxT": ctxT, "lmwT": lmwT, "lmdb": lmdb,
            "EcT": np.ascontiguousarray(EcT_c), "bc": bc_c,
            "embT": np.ascontiguousarray(embT_c), "gmwT": gmwT, "gmdb": gmdb,
            "geT": geT, "gb": gbrow,
            "ones": np.ones((1, 128), np.float32),
        })

    # ---------------- run on the 8 cores ----------------
    nc = _get_program(MP, KP, VP, VW, has_lmdb, has_gmdb, has_lmb, has_gmb)
    res = bass_utils.run_bass_kernel_spmd(
        nc, in_maps, core_ids=list(range(N_CORES)), trace=_trace)

    # ---------------- host assembly ----------------
    lm_logits = np.empty((M, V), np.float32)
    sumexp = np.zeros(M, np.float64)
    for c in range(N_CORES):
        v0, v1 = bounds[c]
        r = res.results[c]
        lm_logits[:, v0:v1] = r["lm_logits"][:M, :v1 - v0]
        se_c = r["lm_sumexp"][:M, 0].astype(np.float64)
        if not has_lmb:
            # pad columns inside the exp width contributed exp(0)=1 each
            se_c = se_c - (VW - (v1 - v0))
        sumexp += se_c
    logZ = np.log(sumexp).astype(np.float32)
    lm_loss = (logZ - lm_logits[np.arange(M), lm_labels]).astype(np.float32)

    gm_logits = np.empty((K2, EV), np.float32)
    gm_lse = np.empty(K2, np.float32)
    for c in range(N_CORES):
        idx = splits[c]
        k = len(idx)
        r = res.results[c]
        if k:
            gm_logits[idx[0]:idx[0] + k] = r["gm_logits"][:k, :EV]
            gm_lse[idx[0]:idx[0] + k] = r["gm_lse"][:k, 0]
    gm_loss = (gm_lse - gm_logits[np.arange(K2), edge_labels] + lasso
               ).astype(np.float32)

    out = (lm_logits, lm_labels, lm_loss, gm_logits, gm_loss, edge_labels)
    if _trace:
        return out, res
    return out


# revision 17
# speedup vs baseline: 1.1763x; 1.0401x over previous
"""Trainium2 Bass kernel for nn_EnhancedMaskDecoder (MLM head + edge/GNN head).

Decomposition across 8 NeuronCores:
  - MLM vocab projection: vocab-sharded (each core owns ~V/8 embedding rows);
    per-shard CE stats (sum of exp) computed on-device, combined on host.
  - Edge path: data-parallel over batch (core c owns batch elements b with
    b % 8 == c and all edges that touch them).  Instead of a dense matmul on
    gathered [K2, 2H] edge embeddings, each core precomputes
    P1 = hidden_b @ W1.T and P2 = hidden_b @ W2.T once (the edge-head weight
    split gm_w = [W1 W2]), then per edge (i,j) the pre-activation is just
    P1[j] + P2[i]: an indirect-DMA gather with a fused add.
  - The small head transform of the masked tokens is replicated on all cores.

Host side (numpy): index extraction, weight folds (LN gain into the
embedding matrices, LN bias into logit biases), sharding/padding, assembly
(concat + CE from per-shard sum-exp stats).  Device: all matmuls, gelu,
layernorm (Newton rsqrt on the vector engine - no act-table swaps),
exp/log-sum-exp.  Matmuls run as float32r (full PE rate, ~2e-4 rel err).
"""
import os
import sys

sys.path.insert(0, "/opt/trn_rl_repo")

import numpy as np

import concourse.bacc as bacc
import concourse.mybir as mybir
import concourse.tile as tile
from concourse import bass_utils
from concourse.bass import IndirectOffsetOnAxis, ds, ts
from concourse.masks import make_identity

N_CORES = 8
H = 1024
LN_EPS = 1e-7
LASSO = 1e-4
NEG = -1e30

F32 = mybir.dt.float32
F32R = mybir.dt.float32r
I32 = mybir.dt.int32
AF = mybir.ActivationFunctionType
ALU = mybir.AluOpType
AX = mybir.AxisListType


def _build_program(MP, KP, VP, VW, LB, has_lmdb, has_gmdb, has_lmb, has_gmb):
    """One SPMD program; all shapes identical across cores.

    VW: exact vocab width for the sum-exp (pad columns beyond VW are never
    exp'd; pad columns inside VW are corrected host-side when has_lmb=False).
    LB: rows of hidden owned per core (ceil(B/8) * L).
    has_*: emit the K=1 ones-row bias matmuls only when the bias is nonzero.
    """
    MPT, KPT, VPT = MP // 128, KP // 128, VP // 512
    H2 = 2 * H
    HC = H // 128
    LBT = LB // 128

    nc = bacc.Bacc("TRN2", target_bir_lowering=False, debug=False)

    ctxT = nc.dram_tensor("ctxT", [H, MP], F32R, kind="ExternalInput")
    lmwT = nc.dram_tensor("lmwT", [H, H], F32R, kind="ExternalInput")
    EcT = nc.dram_tensor("EcT", [H, VP], F32R, kind="ExternalInput")
    hbT = nc.dram_tensor("hbT", [H, LB], F32R, kind="ExternalInput")
    gmwT = nc.dram_tensor("gmwT", [H2, H], F32R, kind="ExternalInput")
    geT = nc.dram_tensor("geT", [H, 64], F32R, kind="ExternalInput")
    jidx = nc.dram_tensor("jidx", [KP, 1], I32, kind="ExternalInput")
    iidx = nc.dram_tensor("iidx", [KP, 1], I32, kind="ExternalInput")
    need_ones = has_lmdb or has_lmb or has_gmb
    if need_ones:
        ones = nc.dram_tensor("ones", [1, 128], F32R, kind="ExternalInput")
    if has_lmdb:
        lmdb = nc.dram_tensor("lmdb", [1, H], F32R, kind="ExternalInput")
    if has_gmdb:
        gmdb = nc.dram_tensor("gmdb", [1, H], F32, kind="ExternalInput")
    if has_lmb:
        bc = nc.dram_tensor("bc", [1, VP], F32R, kind="ExternalInput")
    if has_gmb:
        gb = nc.dram_tensor("gb", [1, 64], F32R, kind="ExternalInput")

    lm_logits = nc.dram_tensor("lm_logits", [MP, VP], F32, kind="ExternalOutput")
    lm_sumexp = nc.dram_tensor("lm_sumexp", [MP, 1], F32, kind="ExternalOutput")
    gm_logits = nc.dram_tensor("gm_logits", [KP, 64], F32, kind="ExternalOutput")
    gm_lse = nc.dram_tensor("gm_lse", [KP, 1], F32, kind="ExternalOutput")

    # per-core scratch: the two halves of the edge-head pre-activation table
    P1d = nc.dram_tensor("P1d", [LB, H], F32)
    P2d = nc.dram_tensor("P2d", [LB, H], F32)

    with tile.TileContext(nc) as tc:
        with tc.tile_pool(name="singles", bufs=1) as singles:
            ident = singles.tile([128, 128], F32)
            make_identity(nc, ident)
            magic = singles.tile([128, 1], I32)
            nc.vector.memset(magic, 0x5F3759DF)
            if need_ones:
                ones1 = singles.tile([1, 128], F32R)
                nc.sync.dma_start(ones1, ones[:, :])
            TT = singles.tile([128, HC, MP], F32R)
            geT_sb = singles.tile([128, HC, 64], F32R)
            nc.sync.dma_start(geT_sb, geT[:, :].rearrange("(c p) v -> p c v", p=128))
            if has_gmb:
                gb_sb = singles.tile([1, 64], F32R)
                nc.sync.dma_start(gb_sb, gb[:, :])
            separts = singles.tile([128, MPT, VPT], F32)

            def newton_rsqrt(pool, rs, v):
                """rs = 1/sqrt(v) via bit-hack seed + Newton steps; DVE-only
                so the scalar engine never swaps its activation-table set."""
                seed = pool.tile(list(rs.shape), F32, tag="nr_seed")
                nc.vector.tensor_scalar(
                    seed.bitcast(I32), v.bitcast(I32), 1, None,
                    op0=ALU.arith_shift_right)
                nc.vector.tensor_tensor(
                    seed.bitcast(I32), magic[: rs.shape[0]], seed.bitcast(I32),
                    mybir.AluOpType.subtract)
                y = rs
                nc.vector.tensor_copy(y, seed)
                a = pool.tile(list(rs.shape), F32, tag="nr_a")
                for _ in range(3):
                    nc.vector.tensor_mul(a, y, y)
                    nc.vector.tensor_mul(a, a, v)
                    nc.vector.tensor_scalar(a, a, -0.5, 1.5,
                                            op0=ALU.mult, op1=ALU.add)
                    nc.vector.tensor_mul(y, y, a)

            def layer_norm_normalize(pool, x_tile):
                """x (128, H) f32 -> (x - mean) * rsqrt(var + eps), in place."""
                nsg = H // 512
                stats = pool.tile([128, nsg, 6], F32, tag="ln_stats")
                for sg in range(nsg):
                    nc.vector.bn_stats(stats[:, sg], x_tile[:, ds(sg * 512, 512)])
                mv = pool.tile([128, 2], F32, tag="ln_mv")
                nc.vector.bn_aggr(mv, stats)
                ve = pool.tile([128, 1], F32, tag="ln_ve")
                nc.vector.tensor_scalar(ve, mv[:, 1:2], LN_EPS, None, op0=ALU.add)
                rs = pool.tile([128, 1], F32, tag="ln_rs")
                newton_rsqrt(pool, rs, ve)
                nc.vector.tensor_scalar(
                    x_tile, x_tile, mv[:, 0:1], rs, op0=ALU.subtract, op1=ALU.mult
                )

            # long-lived: edge-head weights preloaded at program start so the
            # DMA overlaps phase A compute
            with tc.tile_pool(name="gmw", bufs=1) as gmw:
                gmwT_sb = gmw.tile([128, 2 * HC, H], F32R)
                nc.sync.dma_start(
                    gmwT_sb, gmwT[:, :].rearrange("(c p) h -> p c h", p=128))
                glog_all = gmw.tile([128, KPT, 64], F32)
                gse_all = gmw.tile([128, KPT], F32)
                lse_all = gmw.tile([128, KPT], F32)

                # ---------------- Phase A: MLM head transform ----------------
                with tc.tile_pool(name="phA", bufs=3) as pA, \
                     tc.tile_pool(name="phA_ps", bufs=4, space="PSUM") as pAp, \
                     tc.tile_pool(name="phA_pt", bufs=2, space="PSUM") as pAt, \
                     tc.tile_pool(name="phA_w", bufs=1) as pAw:
                    lmwT_sb = pAw.tile([128, HC, H], F32R)
                    nc.sync.dma_start(
                        lmwT_sb, lmwT[:, :].rearrange("(c p) h -> p c h", p=128))
                    if has_lmdb:
                        lmdb_sb = pAw.tile([1, H], F32R)
                        nc.sync.dma_start(lmdb_sb, lmdb[:, :])
                    ctxT_view = ctxT[:, :].rearrange("(c p) m -> p c m", p=128)

                    for mt in range(MPT):
                        ctx_t = pA.tile([128, HC, 128], F32R, tag="ctx_t")
                        nc.sync.dma_start(ctx_t, ctxT_view[:, :, ts(mt, 128)])
                        t_tile = pA.tile([128, H], F32, tag="t_tile")
                        for half in range(2):
                            ps = pAp.tile([128, 512], F32, tag="headps")
                            for k in range(HC):
                                nc.tensor.matmul(
                                    ps, ctx_t[:, k, :],
                                    lmwT_sb[:, k, ds(half * 512, 512)],
                                    start=(k == 0),
                                    stop=(k == HC - 1 and not has_lmdb))
                            if has_lmdb:
                                nc.tensor.matmul(
                                    ps, ones1, lmdb_sb[:, ds(half * 512, 512)],
                                    start=False, stop=True)
                            nc.scalar.activation(
                                t_tile[:, ds(half * 512, 512)], ps, AF.Gelu)
                        layer_norm_normalize(pA, t_tile)
                        for k in range(HC):
                            pt = pAt.tile([128, 128], F32, tag="tps")
                            nc.tensor.transpose(pt, t_tile[:, ts(k, 128)], ident)
                            nc.vector.tensor_copy(TT[:, k, ts(mt, 128)], pt)

                # ------------- Phase B-pre: P1/P2 tables -------------
                with tc.tile_pool(name="phP", bufs=1) as pP, \
                     tc.tile_pool(name="phP_c", bufs=3) as pPc, \
                     tc.tile_pool(name="phP_ps", bufs=4, space="PSUM") as pPp:
                    hbT_sb = pP.tile([128, HC, LB], F32R)
                    nc.sync.dma_start(
                        hbT_sb, hbT[:, :].rearrange("(c p) m -> p c m", p=128))
                    for pm in range(LBT):
                        for Pd, base in ((P1d, 0), (P2d, HC)):
                            for half in range(2):
                                ps = pPp.tile([128, 512], F32, tag="pps")
                                for k in range(HC):
                                    nc.tensor.matmul(
                                        ps, hbT_sb[:, k, ts(pm, 128)],
                                        gmwT_sb[:, base + k, ds(half * 512, 512)],
                                        start=(k == 0), stop=(k == HC - 1))
                                pb_sb = pPc.tile([128, 512], F32, tag="pcopy")
                                nc.any.tensor_copy(pb_sb, ps)
                                nc.sync.dma_start(
                                    Pd[ts(pm, 128), ds(half * 512, 512)], pb_sb)

                # ---------------- Phase B: edge path ----------------
                with tc.tile_pool(name="phB", bufs=3) as pB, \
                     tc.tile_pool(name="phB2", bufs=2) as pB2, \
                     tc.tile_pool(name="phB_w", bufs=1) as pBw, \
                     tc.tile_pool(name="phB_pt", bufs=2, space="PSUM") as pBt, \
                     tc.tile_pool(name="phB_pg", bufs=2, space="PSUM") as pBg:
                    idxj_sb = pBw.tile([128, KPT], I32)
                    nc.sync.dma_start(
                        idxj_sb,
                        jidx[:, :].rearrange("(kt p) one -> p (kt one)", p=128))
                    idxi_sb = pBw.tile([128, KPT], I32)
                    nc.sync.dma_start(
                        idxi_sb,
                        iidx[:, :].rearrange("(kt p) one -> p (kt one)", p=128))
                    if has_gmdb:
                        gmdb_bc = pBw.tile([128, H], F32)
                        import concourse.bass as _bass
                        gmdb_ap = gmdb[:, :]
                        nc.gpsimd.dma_start(
                            gmdb_bc,
                            _bass.AP(tensor=gmdb_ap.tensor, offset=gmdb_ap.offset,
                                     ap=[[0, 128]] + list(gmdb_ap.ap[1:])))

                    for kt in range(KPT):
                        ga = pB.tile([128, H], F32, tag="ga")
                        nc.gpsimd.indirect_dma_start(
                            out=ga[:, :], out_offset=None, in_=P1d[:, :],
                            in_offset=IndirectOffsetOnAxis(
                                ap=idxj_sb[:, kt:kt + 1], axis=0))
                        nc.gpsimd.indirect_dma_start(
                            out=ga[:, :], out_offset=None, in_=P2d[:, :],
                            in_offset=IndirectOffsetOnAxis(
                                ap=idxi_sb[:, kt:kt + 1], axis=0),
                            compute_op=ALU.add)
                        if has_gmdb:
                            nc.vector.tensor_add(ga, ga, gmdb_bc)
                        g_tile = pB2.tile([128, H], F32, tag="g_tile")
                        nc.scalar.activation(g_tile, ga, AF.Gelu)
                        layer_norm_normalize(pB2, g_tile)
                        gT = pB2.tile([128, HC, 128], F32R, tag="gT")
                        for k in range(HC):
                            pt = pBt.tile([128, 128], F32, tag="gps")
                            nc.tensor.transpose(pt, g_tile[:, ts(k, 128)], ident)
                            nc.any.tensor_copy(gT[:, k, :], pt)
                        pg = pBg.tile([128, 64], F32, tag="glps")
                        for k in range(HC):
                            nc.tensor.matmul(pg, gT[:, k, :], geT_sb[:, k, :],
                                             start=(k == 0),
                                             stop=(k == HC - 1 and not has_gmb))
                        if has_gmb:
                            nc.tensor.matmul(pg, ones1, gb_sb,
                                             start=False, stop=True)
                        nc.any.tensor_copy(glog_all[:, kt, :], pg)
                        nc.sync.dma_start(gm_logits[ts(kt, 128), :],
                                          glog_all[:, kt, :])

                    # batched exp + log-sum-exp (single activation-table set)
                    for kt in range(KPT):
                        gexp = pB2.tile([128, 64], F32, tag="gexp")
                        nc.scalar.activation(gexp, glog_all[:, kt, :], AF.Exp,
                                             accum_out=gse_all[:, kt:kt + 1])
                    nc.scalar.activation(lse_all, gse_all, AF.Ln)
                    for kt in range(KPT):
                        nc.sync.dma_start(gm_lse[ts(kt, 128), :],
                                          lse_all[:, kt:kt + 1])

            # ---------------- Phase C: vocab projection ----------------
            with tc.tile_pool(name="phC", bufs=3) as pC, \
                 tc.tile_pool(name="phC_o", bufs=4) as pCo, \
                 tc.tile_pool(name="phC_w", bufs=1) as pCw, \
                 tc.tile_pool(name="phC_ps", bufs=4, space="PSUM") as pCp:
                if has_lmb:
                    bc_sb = pCw.tile([1, VP], F32R)
                    nc.sync.dma_start(bc_sb, bc[:, :])
                EcT_view = EcT[:, :].rearrange("(c p) v -> p c v", p=128)
                for vt in range(VPT):
                    e_t = pC.tile([128, HC, 512], F32R, tag="e_t")
                    nc.sync.dma_start(e_t, EcT_view[:, :, ts(vt, 512)])
                    ew = min(512, VW - vt * 512)
                    for mt in range(MPT):
                        ps = pCp.tile([128, 512], F32, tag="vps")
                        for k in range(HC):
                            nc.tensor.matmul(ps, TT[:, k, ts(mt, 128)],
                                             e_t[:, k, :],
                                             start=(k == 0),
                                             stop=(k == HC - 1 and not has_lmb))
                        if has_lmb:
                            nc.tensor.matmul(ps, ones1,
                                             bc_sb[:, ds(vt * 512, 512)],
                                             start=False, stop=True)
                        lg = pCo.tile([128, 512], F32, tag="lg")
                        nc.vector.tensor_copy(lg, ps)
                        if ew > 0:
                            ex = pCo.tile([128, 512], F32, tag="ex")
                            nc.scalar.activation(
                                ex[:, :ew], ps[:, :ew], AF.Exp,
                                accum_out=separts[:, mt, vt:vt + 1])
                        else:
                            nc.vector.memset(separts[:, mt, vt:vt + 1], 0.0)
                        nc.sync.dma_start(
                            lm_logits[ts(mt, 128), ds(vt * 512, 512)], lg)
                for mt in range(MPT):
                    se = pC.tile([128, 1], F32, tag="se")
                    nc.vector.tensor_reduce(se, separts[:, mt, :], axis=AX.X,
                                            op=ALU.add)
                    nc.sync.dma_start(lm_sumexp[ts(mt, 128), :], se)

    nc.compile()
    return nc


_PROGRAM_CACHE = {}


def _get_program(*key):
    if key not in _PROGRAM_CACHE:
        _PROGRAM_CACHE[key] = _build_program(*key)
    return _PROGRAM_CACHE[key]


def _pad128(n):
    return max(128, -(-n // 128) * 128)


def kernel(hidden, ebd_weight, edge_ebd_weight, lm_dense_w, lm_dense_b, lm_ln_g,
           lm_ln_b, lm_bias, gm_dense_w, gm_dense_b, gm_ln_g, gm_ln_b, gm_bias,
           target_ids, adj_matrix, _trace=False):
    hidden = np.asarray(hidden, np.float32)
    B, L, Hd = hidden.shape
    V = ebd_weight.shape[0]
    EV = edge_ebd_weight.shape[0]
    assert Hd == H

    # ---------------- host prep: MLM ----------------
    flat_t = np.asarray(target_ids).reshape(-1)
    mask_idx = np.nonzero(flat_t > 0)[0]
    M = len(mask_idx)
    MP = _pad128(M)
    flat_h = hidden.reshape(-1, H)
    ctx = np.zeros((MP, H), np.float32)
    ctx[:M] = flat_h[mask_idx]
    lm_labels = flat_t[mask_idx].astype(np.int32)

    vs = -(-V // N_CORES)
    bounds = [(c * vs, min((c + 1) * vs, V)) for c in range(N_CORES)]
    VP = -(-vs // 512) * 512
    VW = vs  # exp width used on device (max shard width)

    E_eff = (np.asarray(ebd_weight, np.float32)
             * np.asarray(lm_ln_g, np.float32)[None, :])
    lm_bias_eff = (np.asarray(lm_bias, np.float32)
                   + np.asarray(ebd_weight, np.float32)
                   @ np.asarray(lm_ln_b, np.float32)).astype(np.float32)

    # ---------------- host prep: edges (sharded by batch element) ----------
    adj = np.asarray(adj_matrix)
    nz = np.stack(np.nonzero(adj), axis=1)
    non_diag = nz[:, 1] != nz[:, 2]
    trans = np.stack([nz[:, 0], nz[:, 2], nz[:, 1]], axis=1)[non_diag]
    nz_all = np.concatenate([nz, trans], axis=0)
    K2 = len(nz_all)
    edge_labels = adj[nz_all[:, 0], nz_all[:, 1], nz_all[:, 2]].astype(np.int32)

    core_of = (nz_all[:, 0] % N_CORES).astype(np.int64)
    blocal = nz_all[:, 0] // N_CORES
    NB = -(-B // N_CORES)
    LB = NB * L
    glob_idx = [np.nonzero(core_of == c)[0] for c in range(N_CORES)]
    KP = _pad128(max(len(g) for g in glob_idx))

    Ge_eff = (np.asarray(edge_ebd_weight, np.float32)
              * np.asarray(gm_ln_g, np.float32)[None, :])
    gm_bias_eff = (np.asarray(gm_bias, np.float32)
                   + np.asarray(edge_ebd_weight, np.float32)
                   @ np.asarray(gm_ln_b, np.float32)).astype(np.float32)

    lasso = np.float32(LASSO * np.abs(np.asarray(gm_dense_w, np.float64)).sum())

    lmdb = np.asarray(lm_dense_b, np.float32).reshape(1, H)
    gmdb = np.asarray(gm_dense_b, np.float32).reshape(1, H)
    has_lmdb = bool(np.any(lmdb))
    has_gmdb = bool(np.any(gmdb))
    has_lmb = bool(np.any(lm_bias_eff))
    has_gmb = bool(np.any(gm_bias_eff)) or EV < 64

    # ---------------- shared uploads ----------------
    ctxT = np.ascontiguousarray(ctx.T)
    lmwT = np.ascontiguousarray(np.asarray(lm_dense_w, np.float32).T)
    gmwT = np.ascontiguousarray(np.asarray(gm_dense_w, np.float32).T)
    geT = np.zeros((H, 64), np.float32)
    geT[:, :EV] = Ge_eff.T
    gbrow = np.full((1, 64), NEG, np.float32)
    gbrow[0, :EV] = gm_bias_eff
    onesrow = np.ones((1, 128), np.float32)

    in_maps = []
    for c in range(N_CORES):
        v0, v1 = bounds[c]
        EcT_c = np.zeros((H, VP), np.float32)
        EcT_c[:, :v1 - v0] = E_eff[v0:v1].T
        bc_c = np.full((1, VP), NEG, np.float32)
        bc_c[0, :v1 - v0] = lm_bias_eff[v0:v1]
        gi = glob_idx[c]
        k = len(gi)
        hb = np.zeros((LB, H), np.float32)
        owned = list(range(c, B, N_CORES))
        for t, b in enumerate(owned):
            hb[t * L:(t + 1) * L] = hidden[b]
        jl = np.zeros((KP, 1), np.int32)
        il = np.zeros((KP, 1), np.int32)
        jl[:k, 0] = blocal[gi] * L + nz_all[gi, 2]
        il[:k, 0] = blocal[gi] * L + nz_all[gi, 1]
        in_maps.append({
            "ctxT": ctxT, "lmwT": lmwT, "lmdb": lmdb,
            "EcT": np.ascontiguousarray(EcT_c), "bc": bc_c,
            "hbT": np.ascontiguousarray(hb.T), "gmwT": gmwT, "gmdb": gmdb,
            "geT": geT, "gb": gbrow, "ones": onesrow,
            "jidx": jl, "iidx": il,
        })

    # ---------------- run on the 8 cores ----------------
    nc = _get_program(MP, KP, VP, VW, LB,
                      has_lmdb, has_gmdb, has_lmb, has_gmb)
    res = bass_utils.run_bass_kernel_spmd(
        nc, in_maps, core_ids=list(range(N_CORES)), trace=_trace)

    # ---------------- host assembly ----------------
    lm_logits = np.empty((M, V), np.float32)
    sumexp = np.zeros(M, np.float64)
    for c in range(N_CORES):
        v0, v1 = bounds[c]
        r = res.results[c]
        lm_logits[:, v0:v1] = r["lm_logits"][:M, :v1 - v0]
        se_c = r["lm_sumexp"][:M, 0].astype(np.float64)
        if not has_lmb:
            se_c = se_c - (VW - (v1 - v0))  # pad cols inside VW gave exp(0)=1
        sumexp += se_c
    logZ = np.log(sumexp).astype(np.float32)
    lm_loss = (logZ - lm_logits[np.arange(M), lm_labels]).astype(np.float32)

    gm_logits = np.empty((K2, EV), np.float32)
    gm_lse = np.empty(K2, np.float32)
    for c in range(N_CORES):
        gi = glob_idx[c]
        k = len(gi)
        r = res.results[c]
        if k:
            gm_logits[gi] = r["gm_logits"][:k, :EV]
            gm_lse[gi] = r["gm_lse"][:k, 0]
    gm_loss = (gm_lse - gm_logits[np.arange(K2), edge_labels] + lasso
               ).astype(np.float32)

    out = (lm_logits, lm_labels, lm_loss, gm_logits, gm_loss, edge_labels)
    if _trace:
        return out, res
    return out
